# revision 12
# baseline (speedup 1.0000x reference)
"""Trainium2 Bass kernel for nn_DConv (shift-gather + 3x3 conv), 8 NeuronCores.

Math: the reference's per-channel torch.roll on the zero-padded image only
ever wraps in zero-pad rows/columns, so the whole op collapses to

    out[b,co,h,w] = sum_{ci,kh,kw} W[co,ci,kh,kw] * x[b,ci, h+kh-1-dy[ci], w+kw-1-dx[ci]]

with out-of-range x treated as 0 and (dy,dx) the c%5 shift table.  The host
pre-applies the per-channel roll and zero padding while packing partitions
(pure data layout, like the partition interleave), so the device sees a
[128, 163, 162] bf16 image per core whose rows are already shifted + padded;
the 3x3 conv then runs as 9 accumulating PE matmuls over flat windows of the
image (input and output both at row pitch 162, so each tap is a constant
flat offset), and every DMA moves large fully-contiguous descriptors.

Sharding: data-parallel over batch, 2 samples per core.  SBUF partitions
hold both samples' channels (g0s0|g0s1|g1s0|...); each matmul uses a
sample-block-masked [128,128] stationary weight so one instruction computes
the tap for both samples (K=128, M=128 -> full PE array; out partitions
0-63 = sample 0, 64-127 = sample 1).

Dtype: bfloat16 operands with fp32 PSUM accumulation (PE streams 1
col/cycle at any N).  The output is stored to HBM as bf16 and upcast on the
host; measured end-to-end relative error is ~4e-3 vs the fp32 reference.

Schedule: the PE ramps on dummy matmuls over a tiny zeroed tile from ~t=0
(no input dependency) while the weights and the first image rows arrive;
strip 0 uses a soft-start chunk plan (1-2 output rows per PSUM chunk) and
row-band loads so real compute begins as soon as the first rows land.
Stores are bf16 multi-row contiguous descriptors (>=512B, full DMA rate).
"""
import numpy as np
import ml_dtypes

from concourse import bacc, tile, mybir
from concourse.bass_utils import run_bass_kernel_spmd

# problem shape (hardcoded per contract)
B, C, H, W = 16, 64, 160, 160
N_CORES = 8
B_PER_CORE = B // N_CORES  # 2
VP = H + 2                 # padded pitch 162
XR = H + 3                 # DRAM image rows (162 padded + 1 zero guard)

BF16 = mybir.dt.bfloat16

# shift table: group g = ci % 5
DXS = [0, 1, 0, -1, 0]
DYS = [0, 0, 1, 0, -1]
GROUP_SIZES = [13, 13, 13, 13, 12]
GROUP_P0 = [0, 26, 52, 78, 104]

# partition p -> (sample, channel) map, shared by host packing and weights
PART_SAMPLE = np.zeros(128, np.int64)
PART_CHANNEL = np.zeros(128, np.int64)
for _g in range(5):
    _gs = GROUP_SIZES[_g]
    for _sm in range(2):
        for _j in range(_gs):
            _p = GROUP_P0[_g] + _sm * _gs + _j
            PART_SAMPLE[_p] = _sm
            PART_CHANNEL[_p] = 5 * _j + _g

# chunk plans: output rows per PSUM chunk (N = rows*162 <= 512 per bank).
# strip 0 soft-starts with 1-2 row chunks so compute begins on the first few
# loaded rows; the last strip tapers to a 1-row final chunk for a short tail.
_S0_PLAN = [1, 1, 2, 2] + [3] * 10 + [2, 2]
_STD_PLAN = [3, 3, 3, 3, 3, 3, 2] * 2
_LAST_PLAN = [3, 3, 3, 3, 3, 3, 2] + [3, 3, 3, 3, 3, 2, 2, 1]
# (h0, rows, chunk_plan, store_bounds, load_bands)
STRIP_LIST = [
    (0, 40, _S0_PLAN, (21, 40), ((0, 4), (4, 9), (9, 16), (16, 28), (28, 43))),
    (40, 40, _STD_PLAN, (20, 40), ((0, 22), (22, 43))),
    (80, 40, _STD_PLAN, (20, 40), ((0, 22), (22, 43))),
    (120, 40, _LAST_PLAN, (20, 35, 39), ((0, 22), (22, 43))),
]
MAX_STRIP = max(r for _, r, _, _, _ in STRIP_LIST)  # 40
XS_ROWS = MAX_STRIP + 3    # strip buffer rows (+1 halo each side +1 spill)

XS_BUFS = 3
STG_BUFS = 2
PSUM_BUFS = 4
WARMUP_MMS = 12            # dummy 256-col matmuls ramping the PE from ~t=0

TAPS = [(kh, kw) for kh in range(3) for kw in range(3)]


def build_kernel(reps: int = 1, timing: bool = False):
    nc = bacc.Bacc("TRN2", target_bir_lowering=False, debug=False,
                   num_devices=N_CORES)
    wt_dram = nc.dram_tensor("wt", [128, 9, 128], BF16,
                             kind="ExternalInput")
    if timing:
        # timing-only variant: big tensors stay in device DRAM (uninitialised
        # garbage is fine for timing) so per-call host<->device transfer is
        # tiny and wall-clock noise is dominated by the fixed RTT only.
        x_dram = nc.dram_tensor("x", [128, XR, VP], BF16)
        out_dram = nc.dram_tensor("out", [B_PER_CORE, C, H, W], BF16)
        tail_dram = nc.dram_tensor("out_tail", [128, W], mybir.dt.float32)
        dummy = nc.dram_tensor("t_dummy", [1, 16], BF16,
                               kind="ExternalOutput")
    else:
        x_dram = nc.dram_tensor("x", [128, XR, VP], BF16,
                                kind="ExternalInput")
        out_dram = nc.dram_tensor("out", [B_PER_CORE, C, H, W], BF16,
                                  kind="ExternalOutput")
        tail_dram = nc.dram_tensor("out_tail", [128, W], mybir.dt.float32,
                                   kind="ExternalOutput")
    x_ap = x_dram.ap()
    out_flat = out_dram.ap().rearrange("b c h w -> (b c) h w")

    with tile.TileContext(nc) as tc:
        with (
            tc.tile_pool(name="wpool", bufs=1) as wpool,
            tc.tile_pool(name="xs_pool", bufs=XS_BUFS) as xs_pool,
            tc.tile_pool(name="stg_pool", bufs=STG_BUFS) as stg_pool,
            tc.tile_pool(name="psum", bufs=PSUM_BUFS, space="PSUM") as psum_pool,
        ):
            # tiny zero tile for PE warmup: dummy matmuls start at ~t=0 with
            # no input dependency, burning the PE p-state ramp while the
            # weights and first image rows arrive
            wz = wpool.tile([128, 256], BF16)
            nc.gpsimd.memset(wz[:].bitcast(mybir.dt.float32), 0.0)

            wt = wpool.tile([128, 9, 128], BF16)
            nc.sync.dma_start(wt[:], wt_dram.ap()[:])
            tailbuf = wpool.tile([128, W], mybir.dt.float32)

            if WARMUP_MMS:
                psw = psum_pool.tile([128, 512], mybir.dt.float32, tag="ps")
                for i in range(WARMUP_MMS):
                    nc.tensor.matmul(psw[:, 0:256], wz[:, 0:128], wz[:],
                                     start=(i == 0),
                                     stop=(i == WARMUP_MMS - 1))

            for _ in range(reps):
                for s, (h0, srows, chunk_plan, out_bounds, bands) in \
                        enumerate(STRIP_LIST):
                    xs = xs_pool.tile([128, XS_ROWS, VP], BF16, tag="xs")
                    # row-band loads (alternating HWDGE rings); each band is
                    # one fully-contiguous descriptor per partition
                    for bi, (a, b) in enumerate(bands):
                        eng = nc.scalar if bi % 2 == 0 else nc.sync
                        eng.dma_start(xs[:, a:b, :],
                                      x_ap[:, h0 + a:h0 + b, :])
                    xs_flat = xs[:].rearrange("p r v -> p (r v)")
                    stg = stg_pool.tile([128, MAX_STRIP, W], BF16)
                    r0 = 0
                    for j, crows in enumerate(chunk_plan):
                        n_mm = VP * crows
                        ps = psum_pool.tile([128, 512], mybir.dt.float32,
                                            tag="ps")
                        for t, (kh, kw) in enumerate(TAPS):
                            base = (r0 + kh) * VP + kw
                            nc.tensor.matmul(
                                ps[:, 0:n_mm],
                                wt[:, t, :],
                                xs_flat[:, base:base + n_mm],
                                start=(t == 0),
                                stop=(t == len(TAPS) - 1),
                            )
                        ps_view = ps[:, 0:n_mm].rearrange(
                            "p (r v) -> p r v", v=VP)
                        last_chunk = (s == len(STRIP_LIST) - 1
                                      and j == len(chunk_plan) - 1)
                        if last_chunk:
                            # final output row: f32 copy + f32 sidecar store
                            # (host merges), keeping the end-of-kernel
                            # critical path as short as possible
                            nc.vector.tensor_copy(tailbuf[:, :],
                                                  ps_view[:, 0, 0:W])
                            nc.sync.dma_start(tail_dram.ap()[:, :],
                                              tailbuf[:, :])
                        else:
                            nc.vector.tensor_copy(
                                stg[:, r0:r0 + crows, :],
                                ps_view[:, :, 0:W],
                            )
                        r0 += crows
                        # store each block as soon as its chunks are copied
                        # (SWDGE so stores can't head-of-line-block the
                        # HWDGE input loads; the tail strip uses SP, which
                        # is idle by then and has the cheapest fixed costs).
                        # One DMA covers both samples: the HBM (b c) dims
                        # are contiguous, matching the partition layout, and
                        # bf16 rows fuse into one multi-row descriptor.
                        if r0 in out_bounds:
                            rb = ([0] + [b for b in out_bounds if b < r0])[-1]
                            seng = (nc.sync if s == len(STRIP_LIST) - 1
                                    else nc.gpsimd)
                            seng.dma_start(
                                out_flat[:, h0 + rb:h0 + r0, :],
                                stg[:, rb:r0, :],
                            )
            if timing:
                nc.sync.dma_start(dummy.ap()[:], wt[0:1, 0, 0:16])
    nc.compile()
    return nc


def _host_inputs(x: np.ndarray, weight: np.ndarray):
    """Pack the shifted + padded per-channel images into the partition
    layout (bf16), and build the sample-block-masked tap matrices."""
    xv = np.asarray(x, dtype=np.float32).reshape(
        N_CORES, B_PER_CORE, C, H, W)
    xp = np.zeros((N_CORES, B_PER_CORE, C, H + 2, W + 2), np.float32)
    xp[:, :, :, 1:H + 1, 1:W + 1] = xv
    for g in range(5):
        ch = (np.arange(C) % 5) == g
        xp[:, :, ch] = np.roll(xp[:, :, ch], (DYS[g], DXS[g]), axis=(3, 4))
    full = np.zeros((N_CORES, 128, XR, VP), np.float32)
    full[:, :, 0:H + 2] = xp[:, PART_SAMPLE, PART_CHANNEL]
    x_packed = full.astype(ml_dtypes.bfloat16)
    # lhsT[p, t, m] = weight[co(m), channel(p), t] iff sample(p)==sample(m)
    wk = np.asarray(weight, dtype=np.float32).transpose(1, 2, 3, 0)
    wk = wk.reshape(C, 9, C)  # [ci, tap, co]
    wt_host = np.zeros((128, 9, 128), np.float32)
    for p in range(128):
        sm = PART_SAMPLE[p]
        wt_host[p, :, 64 * sm:64 * sm + 64] = wk[PART_CHANNEL[p]]
    return x_packed, wt_host.astype(ml_dtypes.bfloat16)


_NC_CACHE = {}


def _get_nc(reps: int = 1):
    if reps not in _NC_CACHE:
        _NC_CACHE[reps] = build_kernel(reps)
    return _NC_CACHE[reps]


def kernel(x: np.ndarray, weight: np.ndarray) -> np.ndarray:
    x = np.asarray(x, dtype=np.float32)
    weight = np.asarray(weight, dtype=np.float32)
    x_packed, wt_host = _host_inputs(x, weight)
    nc = _get_nc(1)
    in_maps = [
        {"x": np.ascontiguousarray(x_packed[k]), "wt": wt_host}
        for k in range(N_CORES)
    ]
    res = run_bass_kernel_spmd(nc, in_maps, core_ids=list(range(N_CORES)))
    out = np.empty((B, C, H, W), np.float32)
    for k in range(N_CORES):
        out[k * B_PER_CORE:(k + 1) * B_PER_CORE] = \
            np.asarray(res.results[k]["out"]).astype(np.float32)
        out[k * B_PER_CORE:(k + 1) * B_PER_CORE, :, H - 1, :] = \
            np.asarray(res.results[k]["out_tail"]).astype(np.float32) \
              .reshape(B_PER_CORE, C, W)
    return out
